# revision 27
# baseline (speedup 1.0000x reference)
"""SAGAN-style attention block on 8 trn2 NeuronCores, batch-parallel.

Math per batch element (C=64, H=W=64, S=4096, T=S/4=1024):
  theta = w_theta @ x                      [8, S]
  phi   = maxpool2(w_phi @ x)              [8, T]
  g     = maxpool2(w_g @ x)                [32, T]
  beta  = softmax_t(theta^T @ phi)         [S, T]
  out   = gamma * (w_o @ (g @ beta^T)) + x [C, S]

Wall-clock of a call is dominated by the axon tunnel (measured: ~83ms
request round-trip latency, ~115MB/s host->device, ~34MB/s
device->host), NOT device exec (~100us simulated). The kernel is
therefore shaped to minimize wire traffic and round trips:
  - x ships fp8e4m3 (4.2MB instead of 16.8MB f32); the residual is
    added host-side from the exact f32 x, so fp8 noise only enters the
    attention branch, which is scaled by gamma=0.1 (end-to-end rel err
    8.7e-3 vs the 2e-2 gate)
  - the device returns only the normalized pre-w_o attention tensor
    o2 = (g @ beta^T)/Z, quantized to int4 with a per-(row, 512-col
    block) f32 scale and nibble-packed (1.07MB on the wire instead of
    2.1MB fp8; D2H measured ~3x slower than H2D). End-to-end rel err
    1.04e-2 vs the 2e-2 gate (8.7e-3 with fp8 o2). The tiny w_o
    matmul, gamma scale and residual add run host-side
  - per-core results are AllGathered on-device so the host pulls ONE
    shard from core 0 instead of eight per-core shards (each extra
    D2H pull costs most of a tunnel round trip)
  - the PJRT executable is AOT-compiled ONCE and cached; the prior
    version re-traced + re-lowered + re-shipped the NEFF every call
    (~120ms/call). fast_dispatch_compile removes the effects-token
    sync so dispatch is the C++ fast path (~5ms)
  - host fp8->f32 decode goes through a 256-entry LUT (numpy gather),
    ~1.6x faster than ml_dtypes astype on this 1-vCPU host

Device schedule (per core, 2 batch elements; ~90us simulated, ACT
exp-roofline-bound — see _body comments).
"""

import os
import sys

import numpy as np

os.environ.setdefault("JAX_PLATFORMS", "axon,cpu")
# smaller NEFF to ship on first compile (debug info is never read here)
os.environ.setdefault("CONCOURSE_SCRUB_NEFF_DEBUG_INFO", "1")
for _p in ("/opt/trn_rl_repo",):
    if _p not in sys.path:
        sys.path.insert(0, _p)

import jax
import concourse.bacc as bacc
import concourse.tile as tile
from concourse import mybir
from concourse import bass2jax

F32 = mybir.dt.float32
BF16 = mybir.dt.bfloat16
F8 = mybir.dt.float8e4
U8 = mybir.dt.uint8
AX = mybir.AluOpType
EXP = mybir.ActivationFunctionType.Exp
BF16_NP = mybir.dt.np(mybir.dt.bfloat16)
F8_NP = mybir.dt.np(mybir.dt.float8e4)

N_CORES = 8
NB = 2          # batch elements per core
C = 64
S = 4096        # H*W
T = 1024        # pooled spatial
SB = 512        # s-block width
NSB = S // SB   # 8
NTC = T // 128  # 8 t-chunks
GROUPS = [(0, 2), (2, 5), (5, 8)]  # t-chunk grouping for big ACT exp ops
PB = SB // 2      # packed bytes per s-block (2 int4 / byte)
Q = 7.49          # int4 quant: u = round(o2*Q/amax + Q) in [0, 15]
ROW = NSB * PB + 4 * NSB  # 2048 packed bytes + 8 f32 scales per row

_cache = {}
last_results = None

# C helpers for the two host-side hot loops (1 vCPU, numpy is ~4x slower):
# f32 -> fp8e4m3 input cast (F16C convert + 64K LUT) and int4 output decode
# (byte -> two scaled f32). Compiled on first use; numpy fallback if cc or
# the compile is unavailable.
_C_SRC = r"""
#include <stdint.h>
#include <immintrin.h>

void cast_f32_to_f8(const float *x, const uint8_t *lut, uint8_t *out,
                    long n) {
    long i = 0;
    for (; i + 8 <= n; i += 8) {
        __m256 v = _mm256_loadu_ps(x + i);
        __m128i h = _mm256_cvtps_ph(v, _MM_FROUND_TO_NEAREST_INT);
        uint16_t tmp[8];
        _mm_storeu_si128((__m128i *)tmp, h);
        out[i + 0] = lut[tmp[0]];
        out[i + 1] = lut[tmp[1]];
        out[i + 2] = lut[tmp[2]];
        out[i + 3] = lut[tmp[3]];
        out[i + 4] = lut[tmp[4]];
        out[i + 5] = lut[tmp[5]];
        out[i + 6] = lut[tmp[6]];
        out[i + 7] = lut[tmp[7]];
    }
    for (; i < n; i++) {
        uint16_t h = _cvtss_sh(x[i], _MM_FROUND_TO_NEAREST_INT);
        out[i] = lut[h];
    }
}

/* raw: rows x rowbytes, each row = nblk*pb packed bytes then nblk f32
   amax scales; lutpair: 256 pairs of (hi - Q, lo - Q); out: rows x
   (nblk*pb*2) floats, scaled by amax/Q per block. */
void decode_int4(const uint8_t *raw, const float *lutpair, float *out,
                 long rows, long nblk, long pb, long rowbytes, float inv_q) {
    for (long r = 0; r < rows; r++) {
        const uint8_t *prow = raw + r * rowbytes;
        const float *amax = (const float *)(prow + nblk * pb);
        float *orow = out + r * nblk * pb * 2;
        for (long j = 0; j < nblk; j++) {
            float s = amax[j] * inv_q;
            const uint8_t *p = prow + j * pb;
            float *o = orow + j * pb * 2;
            for (long i = 0; i < pb; i++) {
                const float *pair = lutpair + 2 * p[i];
                o[2 * i] = pair[0] * s;
                o[2 * i + 1] = pair[1] * s;
            }
        }
    }
}

/* Fused int4 decode -> (w_og @ o2) -> + x residual.
   raw: [B][CH][rowbytes] device output (packed int4 + per-block scales)
   w_og: [OC][CH], x/out: [B][OC][nblk*pb*2] f32. out = w_og@o2 + x. */
void post_all(const uint8_t *raw, const float *lutpair, const float *w_og,
              const float *x, float *out, long B, long CH, long OC,
              long nblk, long pb, long rowbytes, float inv_q) {
    long S = nblk * pb * 2;
    long bw = pb * 2; /* block width in floats (1024 halves? no: pb*2) */
    float vals[32 * 1024] __attribute__((aligned(32)));
    for (long b = 0; b < B; b++) {
        const uint8_t *rb = raw + b * CH * rowbytes;
        for (long j = 0; j < nblk; j++) {
            for (long c = 0; c < CH; c++) {
                const uint8_t *prow = rb + c * rowbytes;
                const float *amax = (const float *)(prow + nblk * pb);
                float s = amax[j] * inv_q;
                const uint8_t *p = prow + j * pb;
                float *v = vals + c * bw;
                for (long i = 0; i < pb; i++) {
                    const float *pair = lutpair + 2 * p[i];
                    v[2 * i] = pair[0] * s;
                    v[2 * i + 1] = pair[1] * s;
                }
            }
            for (long o = 0; o < OC; o += 4) {
                const float *w0 = w_og + o * CH;
                const float *w1 = w_og + (o + 1) * CH;
                const float *w2 = w_og + (o + 2) * CH;
                const float *w3 = w_og + (o + 3) * CH;
                const float *xr = x + (b * OC + o) * S + j * bw;
                float *orow = out + (b * OC + o) * S + j * bw;
#ifdef __AVX512F__
                for (long n = 0; n < bw; n += 16) {
                    __m512 a0 = _mm512_loadu_ps(xr + n);
                    __m512 a1 = _mm512_loadu_ps(xr + S + n);
                    __m512 a2 = _mm512_loadu_ps(xr + 2 * S + n);
                    __m512 a3 = _mm512_loadu_ps(xr + 3 * S + n);
                    for (long c = 0; c < CH; c++) {
                        __m512 v = _mm512_loadu_ps(vals + c * bw + n);
                        a0 = _mm512_fmadd_ps(_mm512_set1_ps(w0[c]), v, a0);
                        a1 = _mm512_fmadd_ps(_mm512_set1_ps(w1[c]), v, a1);
                        a2 = _mm512_fmadd_ps(_mm512_set1_ps(w2[c]), v, a2);
                        a3 = _mm512_fmadd_ps(_mm512_set1_ps(w3[c]), v, a3);
                    }
                    _mm512_storeu_ps(orow + n, a0);
                    _mm512_storeu_ps(orow + S + n, a1);
                    _mm512_storeu_ps(orow + 2 * S + n, a2);
                    _mm512_storeu_ps(orow + 3 * S + n, a3);
                }
#else
                for (long n = 0; n < bw; n += 8) {
                    __m256 a0 = _mm256_loadu_ps(xr + n);
                    __m256 a1 = _mm256_loadu_ps(xr + S + n);
                    __m256 a2 = _mm256_loadu_ps(xr + 2 * S + n);
                    __m256 a3 = _mm256_loadu_ps(xr + 3 * S + n);
                    for (long c = 0; c < CH; c++) {
                        __m256 v = _mm256_loadu_ps(vals + c * bw + n);
                        a0 = _mm256_fmadd_ps(_mm256_set1_ps(w0[c]), v, a0);
                        a1 = _mm256_fmadd_ps(_mm256_set1_ps(w1[c]), v, a1);
                        a2 = _mm256_fmadd_ps(_mm256_set1_ps(w2[c]), v, a2);
                        a3 = _mm256_fmadd_ps(_mm256_set1_ps(w3[c]), v, a3);
                    }
                    _mm256_storeu_ps(orow + n, a0);
                    _mm256_storeu_ps(orow + S + n, a1);
                    _mm256_storeu_ps(orow + 2 * S + n, a2);
                    _mm256_storeu_ps(orow + 3 * S + n, a3);
                }
#endif
            }
        }
    }
}
"""


def _build_chelper():
    import ctypes
    import subprocess
    import tempfile

    try:
        d = tempfile.mkdtemp(prefix="k_chelp_")
        src = os.path.join(d, "helper.c")
        so = os.path.join(d, "helper.so")
        with open(src, "w") as f:
            f.write(_C_SRC)
        subprocess.run(
            ["cc", "-O3", "-march=native", "-shared", "-fPIC", "-o", so, src],
            check=True, capture_output=True, timeout=120,
        )
        lib = ctypes.CDLL(so)
        lib.cast_f32_to_f8.argtypes = [
            ctypes.c_void_p, ctypes.c_void_p, ctypes.c_void_p, ctypes.c_long]
        lib.decode_int4.argtypes = [
            ctypes.c_void_p, ctypes.c_void_p, ctypes.c_void_p,
            ctypes.c_long, ctypes.c_long, ctypes.c_long, ctypes.c_long,
            ctypes.c_float]
        lib.post_all.argtypes = [
            ctypes.c_void_p, ctypes.c_void_p, ctypes.c_void_p, ctypes.c_void_p,
            ctypes.c_void_p, ctypes.c_long, ctypes.c_long, ctypes.c_long,
            ctypes.c_long, ctypes.c_long, ctypes.c_long, ctypes.c_float]
        return lib
    except Exception:
        return None


def _build_program():
    nc = bacc.Bacc(None, target_bir_lowering=False, debug=False, num_devices=N_CORES)
    xin = nc.dram_tensor("xin", [NB, C, S], F8, kind="ExternalInput")
    # cols 0:96 = fused conv weights; rows 0:32 of cols 96:128 = identity
    wcat = nc.dram_tensor("wcat", [C, 128], BF16, kind="ExternalInput")
    # per row: 2048 bytes of nibble-packed int4 o2 + 8 f32 block scales
    yout = nc.dram_tensor("yout", [N_CORES, NB, 32, ROW], U8, kind="ExternalOutput")

    with tile.TileContext(nc) as tc:
        with nc.allow_low_precision(reason="bf16 attention; residual is f32 host-side"):
            _body(tc, xin, wcat, yout)
    nc.compile()
    return nc


def _body(tc, xin, wcat, yout):
    nc = tc.nc
    with (
        tc.tile_pool(name="const", bufs=1) as cpool,
        tc.tile_pool(name="big", bufs=2) as bpool,
        tc.tile_pool(name="work", bufs=2) as wpool,
        tc.tile_pool(name="stexp", bufs=4) as epool,
        tc.tile_pool(name="dram", bufs=1, space="DRAM") as dpool,
        tc.psum_pool(name="ps_sc", bufs=2) as ps_sc,
        tc.psum_pool(name="ps_o", bufs=2) as ps_o,
    ):
        # per-core result staged in internal DRAM, AllGathered to every
        # core's ExternalOutput so the host fetches ONE shard instead of
        # eight per-core shards (each extra D2H pull costs ~a tunnel
        # roundtrip)
        ylocal = dpool.tile([NB, 32, ROW], U8)
        ybounce = dpool.tile([N_CORES, NB, 32, ROW], U8)
        wcat_sb = cpool.tile([C, 128], BF16)
        nc.sync.dma_start(wcat_sb[:], wcat[:])
        ident_sb = wcat_sb[0:32, 96:128]
        ones_f = cpool.tile([128, 1], F32)
        nc.vector.memset(ones_f[:], 1.0)
        # warm-up exp on a scalar so the framework emits LoadActFuncSet at
        # the head of the ACT queue (during the input DMA) instead of lazily
        # right before the first real exp ~8us in
        act_warm = cpool.tile([1, 1], F32)
        nc.scalar.activation(act_warm[:], ones_f[0:1, 0:1], EXP)

        # dummy custom-DVE op (output unused): routes DVE table generation
        # through the process-cached dve_table_for_ops path (~0.3s/compile
        # saved). Emitted via a closure after batch 0's conv so it does not
        # sit at the head of the DVE queue.
        def dve_dummy_op():
            dve_dummy = cpool.tile([1, 1], F32)
            nc.vector.reciprocal_approx_fast(dve_dummy[:], ones_f[0:1, 0:1])

        state = {}

        def p1_start(b):
            """input DMA (group-aligned slices) + fp8->bf16 casts + tile
            allocation for batch b. Cast g follows its own DMA slice; casts
            alternate DVE/GpSimd so no cast is queue-blocked by copies."""
            x8_sb = bpool.tile([C, S], F8, tag="x8")
            x_sb = bpool.tile([C, S], BF16, tag="x")
            pre_sb = bpool.tile([96, S], BF16, tag="pre")
            phm = wpool.tile([8, 2048], BF16, tag="phm")
            phi_sb = wpool.tile([8, T], BF16, tag="phi")
            for gi, (g0, g1) in enumerate(GROUPS):
                nc.sync.dma_start(
                    x8_sb[:, g0 * SB:g1 * SB], xin[b][:, g0 * SB:g1 * SB])
                # batch 1's casts all ride GpSimd: its phi chain competes
                # with batch 0's steady-state DVE work (recip/mult)
                eng = nc.gpsimd if (b == 1 or gi == 1) else nc.vector
                eng.tensor_copy(
                    x_sb[:, g0 * SB:g1 * SB], x8_sb[:, g0 * SB:g1 * SB])
            scales_sb = bpool.tile([32, NSB], F32, tag="scales")
            state[b] = {"x8": x8_sb, "x": x_sb, "pre": pre_sb, "phm": phm,
                        "phi": phi_sb, "scales": scales_sb}

        def p1_conv(b, gi):
            """conv group gi for batch b: 3 matmuls -> copies.
            theta+phi rows copy on DVE (feeds pools/scores); g rows on GpSimd.
            The h-direction phi maxpool runs per group right after its copy."""
            st = state[b]
            x_sb, pre_sb, phm = st["x"], st["pre"], st["phm"]
            g0, g1 = GROUPS[gi]
            cps = ps_sc.tile([96, (g1 - g0) * SB], F32, tag="sc")
            for j in range(g0, g1):
                nc.tensor.matmul(
                    cps[:, (j - g0) * SB:(j - g0 + 1) * SB],
                    wcat_sb[:, 0:96], x_sb[:, j * SB:(j + 1) * SB],
                    start=True, stop=True,
                )
            if b == 0:
                # b0: phi-critical rows drain on DVE, g rows on ACT (idle
                # during startup; GpSimd can't read PSUM) so g2t can start
                # early
                nc.vector.tensor_copy(
                    pre_sb[0:40, g0 * SB:g1 * SB], cps[0:40, :])
                nc.scalar.activation(
                    pre_sb[64:96, g0 * SB:g1 * SB], cps[64:96, :],
                    mybir.ActivationFunctionType.Copy)
            else:
                # b1: one full-width DVE drain (same free-dim cost as the
                # 40-row copy) keeps its Copies out of the mid-stream ACT
                # queue; b1's g-path has slack so nothing needs them early
                nc.vector.tensor_copy(
                    pre_sb[:, g0 * SB:g1 * SB], cps[:])
            # phi h-max for this group's columns (cols are (h w) pairs in w)
            nrow = (g1 - g0) * SB // 64  # 64-wide w rows in this slice
            pv = pre_sb[32:40, g0 * SB:g1 * SB].rearrange(
                "p (h w) -> p h w", h=nrow)
            nc.vector.tensor_tensor(
                phm[:, g0 * SB // 2:g1 * SB // 2].rearrange(
                    "p (h w) -> p h w", h=nrow),
                pv[:, :, 0:64:2], pv[:, :, 1:64:2], AX.max)
            # phi w-max for the same slice: yields phi t-chunks [g0*128,
            # g1*128), exactly the score chunks this group's exps will read,
            # so the first scores can start after conv group 0 alone
            phi_sb = st["phi"]
            ph2 = phm[:, g0 * SB // 2:g1 * SB // 2].rearrange(
                "p (h w) -> p h w", h=nrow)
            nc.vector.tensor_tensor(
                phi_sb[:, g0 * 128:g1 * 128].rearrange(
                    "p (h w) -> p h w", h=nrow // 2),
                ph2[:, 0:nrow:2, :], ph2[:, 1:nrow:2, :], AX.max)

        def p1_gpools(b):
            """g maxpool (DVE) — only gates g2t, emitted off the scores path."""
            st = state[b]
            pre_sb = st["pre"]
            g_sb = wpool.tile([32, T], BF16, tag="g")
            ghm = wpool.tile([32, 2048], BF16, tag="ghm")
            gv = pre_sb[64:96].rearrange("p (h w) -> p h w", h=64)
            nc.vector.tensor_tensor(
                ghm[:].rearrange("p (h w) -> p h w", h=64),
                gv[:, :, 0:64:2], gv[:, :, 1:64:2], AX.max)
            gh2 = ghm[:].rearrange("p (h w) -> p h w", h=64)
            nc.vector.tensor_tensor(
                g_sb[:].rearrange("p (h w) -> p h w", h=32),
                gh2[:, 0:64:2, :], gh2[:, 1:64:2, :], AX.max)
            st["g"] = g_sb

        def phase1_g2t(b):
            """g2T chunks: [128 t, 33] = g[:, chunk].T via identity; col 32 =
            ones. Emitted after the first scores block of batch b so the PE
            queue starts scores as soon as phi is pooled."""
            g_sb = state[b]["g"]
            g2t_sb = bpool.tile([128, NTC * 33], BF16, tag="g2t")
            nc.gpsimd.tensor_copy(
                g2t_sb[:].rearrange("p (k c) -> p k c", c=33)[:, :, 32],
                ones_f[:].to_broadcast([128, NTC]))
            for k in range(NTC):
                g2ps = ps_o.tile([128, 32], F32, tag="o")
                nc.tensor.matmul(
                    g2ps[:], g_sb[:, k * 128:(k + 1) * 128], ident_sb[:],
                    start=True, stop=True,
                )
                nc.vector.tensor_copy(g2t_sb[:, k * 33:k * 33 + 32], g2ps[:])
            state[b]["g2t"] = g2t_sb

        def p2_scores(j, b):
            """scores -> exp for (j, b). One st_exp tile per exp group so
            the o-matmul's per-chunk reads depend only on their own group's
            exp, not all three."""
            pre_sb, phi_sb = state[b]["pre"], state[b]["phi"]
            theta = pre_sb[0:8]
            st_exp = []
            for gi, (k0, k1) in enumerate(GROUPS):
                scps = ps_sc.tile([128, 3 * SB], F32, tag="sc")
                for k in range(k0, k1):
                    nc.tensor.matmul(
                        scps[:, (k - k0) * SB:(k - k0 + 1) * SB],
                        phi_sb[:, k * 128:(k + 1) * 128],
                        theta[:, j * SB:(j + 1) * SB],
                        start=True, stop=True,
                    )
                se = epool.tile([128, (k1 - k0) * SB], BF16, tag=f"se{gi}")
                nc.scalar.activation(se[:], scps[:, 0:(k1 - k0) * SB], EXP)
                st_exp.append(se)
            return st_exp

        def p2_rest(j, b, st_exp):
            """o-matmul -> normalize -> int4 quantize+pack -> DMA of (j, b)."""
            g2t_sb = state[b]["g2t"]
            o_ps = ps_o.tile([33, SB], F32, tag="o")
            for k in range(NTC):
                gi = 0 if k < 2 else (1 if k < 5 else 2)
                kk = k - GROUPS[gi][0]
                nc.tensor.matmul(
                    o_ps[:],
                    g2t_sb[:, k * 33:(k + 1) * 33],
                    st_exp[gi][:, kk * SB:(kk + 1) * SB],
                    start=(k == 0), stop=(k == NTC - 1),
                )

            # normalize straight out of PSUM (no staging copy): the "o" ring
            # slot stays held until the mult reads it, which is still well
            # before the next-but-one o-matmul needs the bank. 1/Z fans out
            # across the 32 channel partitions on the GpSimd engine so the
            # mult has a single PSUM operand.
            zr = wpool.tile([1, SB], BF16, tag="zr")
            nc.vector.reciprocal(zr[:], o_ps[32:33, :])
            zb_sb = wpool.tile([32, SB], BF16, tag="zb")
            nc.gpsimd.partition_broadcast(zb_sb[:], zr[:])
            o_f = wpool.tile([32, SB], F32, tag="of")
            nc.vector.tensor_tensor(o_f[:], o_ps[0:32, :], zb_sb[:], AX.mult)
            # int4 quantize with per-(row, block) scale: u = o*Q/amax + Q
            # rounds into [0, 15]; amax=0 rows decode to 0 via the host-side
            # amax multiply, so no special-casing beyond the 1e-6 clamp
            amax = wpool.tile([32, 1], F32, tag="amax")
            nc.vector.tensor_reduce(
                amax[:], o_f[:], mybir.AxisListType.X, AX.max,
                apply_absolute_value=True)
            nc.vector.tensor_scalar_max(amax[:], amax[:], 1e-6)
            rcp = wpool.tile([32, 1], F32, tag="rcp")
            nc.vector.reciprocal(rcp[:], amax[:])
            rsc = wpool.tile([32, 1], F32, tag="rsc")
            nc.vector.tensor_scalar_mul(rsc[:], rcp[:], Q)
            u8 = wpool.tile([32, SB], U8, tag="u8")
            nc.scalar.activation(
                u8[:], o_f[:], mybir.ActivationFunctionType.Copy,
                bias=Q, scale=rsc[:])
            # nibble-pack adjacent columns: byte i = u[2i]*16 + u[2i+1]
            hi = wpool.tile([32, PB], U8, tag="hi")
            nc.vector.tensor_scalar_mul(hi[:], u8[:, 0:SB:2], 16)
            pk = wpool.tile([32, PB], U8, tag="pk")
            nc.vector.tensor_tensor(pk[:], hi[:], u8[:, 1:SB:2], AX.add)
            nc.sync.dma_start(ylocal[b][:, j * PB:(j + 1) * PB], pk[:])
            nc.vector.tensor_copy(state[b]["scales"][:, j:j + 1], amax[:])

        # staggered schedule: batch 0's first scores start as early as
        # possible; g2t and batch 1's conv groups ride in the exp shadow of
        # batch 0's early j-blocks; then (j, b) pairs alternate so
        # PE/ACT/DVE/GpSimd stay fed
        p1_start(0)
        for gi in range(3):
            p1_conv(0, gi)
        se00 = p2_scores(0, 0)
        dve_dummy_op()
        p1_start(1)
        p1_conv(1, 0)
        se10 = p2_scores(1, 0)
        p1_conv(1, 1)
        p1_conv(1, 2)
        p1_gpools(0)
        phase1_g2t(0)
        p2_rest(0, 0, se00)
        se20 = p2_scores(2, 0)
        p1_gpools(1)
        p2_rest(1, 0, se10)
        phase1_g2t(1)

        order = [(0, 1)]
        for j in range(3, NSB):
            order.append((j, 0))
            order.append((j - 2, 1))
        order.append((NSB - 2, 1))
        order.append((NSB - 1, 1))
        # two-deep software pipeline: scores run ahead of the o-matmuls so
        # the PE queue always has the next blocks' scores ready, keeping
        # ACT's exp stream gapless (st_exp rings hold the blocks in flight)
        from collections import deque
        pend = deque([(2, 0, se20)])
        for (j, b) in order[:-1]:
            se = p2_scores(j, b)
            pend.append((j, b, se))
            if len(pend) > 2:
                p2_rest(*pend.popleft())
        jl, bl = order[-1]
        sel = p2_scores(jl, bl)
        while pend:
            p2_rest(*pend.popleft())
        p2_rest(jl, bl, sel)

        # per-batch block scales ride in-band after the packed bytes
        for b in range(NB):
            nc.sync.dma_start(
                ylocal[b][:, NSB * PB:ROW], state[b]["scales"][:].bitcast(U8))

        # gather every core's result so core 0 holds the full batch
        nc.gpsimd.collective_compute(
            "AllGather",
            mybir.AluOpType.bypass,
            replica_groups=[list(range(N_CORES))],
            ins=[ylocal.opt()],
            outs=[ybounce.opt()],
        )
        nc.sync.dma_start(yout[:], ybounce[:])


def _build_executable():
    """AOT-compile the sharded PJRT executable once.

    Bypasses run_bass_kernel_spmd, which re-traces, re-lowers and re-ships
    the NEFF on every call (~120ms/call through the axon tunnel). The
    donated zero output buffers it uploads each call are also dropped: the
    kernel writes every element of yout, so uninitialized custom-call
    result buffers are fine.
    """
    from jax.sharding import Mesh, PartitionSpec
    from jax.experimental.shard_map import shard_map

    nc = _build_program()
    bass2jax.install_neuronx_cc_hook()
    partition_name = nc.partition_id_tensor.name if nc.partition_id_tensor else None
    out_aval = jax.core.ShapedArray((N_CORES, NB, 32, ROW), np.uint8)
    in_names = ["xin", "wcat"] + ([partition_name] if partition_name else [])

    def _exec_body(xin, wcat):
        operands = [xin, wcat]
        if partition_name is not None:
            operands.append(bass2jax.partition_id_tensor())
        outs = bass2jax._bass_exec_p.bind(
            *operands,
            out_avals=(out_aval,),
            in_names=tuple(in_names),
            out_names=("yout",),
            lowering_input_output_aliases=(),
            sim_require_finite=True,
            sim_require_nnan=True,
            nc=nc,
        )
        return outs[0]

    devices = jax.devices()[:N_CORES]
    mesh = Mesh(np.asarray(devices), ("core",))
    sharded = shard_map(
        _exec_body,
        mesh=mesh,
        in_specs=(PartitionSpec("core"), PartitionSpec("core")),
        # the on-device AllGather makes yout identical on every core; P()
        # marks it replicated so np.asarray pulls from a single shard
        out_specs=PartitionSpec(),
        check_rep=False,
    )
    xin_tmpl = jax.ShapeDtypeStruct((N_CORES * NB, C, S), F8_NP)
    wcat_tmpl = jax.ShapeDtypeStruct((N_CORES * C, 128), BF16_NP)
    return bass2jax.fast_dispatch_compile(
        lambda: jax.jit(sharded).lower(xin_tmpl, wcat_tmpl).compile()
    )


def _get_cached():
    if "exe" not in _cache:
        _cache["exe"] = _build_executable()
        # packed byte -> (hi, lo) int4 value pairs, bias pre-subtracted; the
        # numpy gather is the fastest decode on this 1-vCPU host
        b = np.arange(256, dtype=np.uint8)
        _cache["lut4"] = np.stack(
            [(b >> 4).astype(np.float32) - Q, (b & 15).astype(np.float32) - Q],
            axis=1,
        )
        # f16 -> fp8e4m3 cast table: f32->f16 (SIMD) + byte gather is a bit
        # faster than ml_dtypes' direct f32->fp8 on this host; the rare
        # double-rounding ties (0.4% of values, 1 ulp) are noise here
        _cache["lut_f8"] = (
            np.arange(65536, dtype=np.uint16).view(np.float16)
            .astype(np.float32).astype(F8_NP)
        )
        _cache["clib"] = _build_chelper()
    return _cache["exe"], _cache["lut4"], _cache["lut_f8"], _cache["clib"]


def kernel(x, w_theta, w_phi, w_g, w_o, gamma):
    global last_results
    last_results = None
    exe, lut4, lut_f8, clib = _get_cached()

    x = np.ascontiguousarray(np.asarray(x, dtype=np.float32)).reshape(16, C, S)
    if clib is not None:
        x_f8 = np.empty((16, C, S), F8_NP)
        clib.cast_f32_to_f8(
            x.ctypes.data, lut_f8.ctypes.data, x_f8.ctypes.data, x.size)
    else:
        x_f8 = lut_f8[x.astype(np.float16).view(np.uint16)]

    wcat_full = np.zeros((128, C), dtype=np.float32)
    wcat_full[0:8] = np.asarray(w_theta)
    wcat_full[32:40] = np.asarray(w_phi)
    wcat_full[64:96] = np.asarray(w_g)
    wcat_full[96:128, 0:32] = np.eye(32, dtype=np.float32)
    wcat_1 = np.ascontiguousarray(wcat_full.T).astype(BF16_NP)
    wcat_np = np.ascontiguousarray(
        np.broadcast_to(wcat_1, (N_CORES, C, 128))
    ).reshape(N_CORES * C, 128)

    out = exe(x_f8, wcat_np)
    # pull the single replicated shard (one D2H round trip)
    raw = np.asarray(out.addressable_shards[0].data).reshape(16, 32, ROW)

    # decode int4 o2 (byte i of block j -> cols (2i, 2i+1); scale per
    # block), then out = gamma*(w_o @ o2) + x
    w_og = np.ascontiguousarray(
        (float(np.asarray(gamma)) * np.asarray(w_o)).astype(np.float32))
    if clib is not None:
        res = np.empty((16, C, S), np.float32)
        clib.post_all(
            raw.ctypes.data, lut4.ctypes.data, w_og.ctypes.data,
            x.ctypes.data, res.ctypes.data, 16, 32, C, NSB, PB, ROW, 1.0 / Q)
    else:
        amax = np.ascontiguousarray(raw[:, :, NSB * PB:]).view(np.float32)
        o2f = lut4[raw[:, :, :NSB * PB]].reshape(16, 32, NSB, SB)
        o2f *= (amax * (1.0 / Q))[..., None]
        res = np.matmul(w_og, o2f.reshape(16, 32, S))
        res += x
    return res.reshape(16, C, 64, 64)


# revision 30
# speedup vs baseline: 1.0607x; 1.0607x over previous
"""SAGAN-style attention block on 8 trn2 NeuronCores, batch-parallel.

Math per batch element (C=64, H=W=64, S=4096, T=S/4=1024):
  theta = w_theta @ x                      [8, S]
  phi   = maxpool2(w_phi @ x)              [8, T]
  g     = maxpool2(w_g @ x)                [32, T]
  beta  = softmax_t(theta^T @ phi)         [S, T]
  out   = gamma * (w_o @ (g @ beta^T)) + x [C, S]

Wall-clock of a call is dominated by the axon tunnel (measured: ~83ms
request round-trip latency, ~115MB/s host->device, ~55MB/s
device->host; a D2H pull costs its own round trip on top of the
execute's), NOT device exec (~100us simulated). The call's serial
chain is cast -> dispatch -> [upload 4.2MB | exec | ready round trip |
pull 1.07MB] -> host post, ~170ms total, of which ~166ms is protocol
floor (2 RTTs + wire bytes). The kernel is shaped accordingly:
  - x ships fp8e4m3 (4.2MB instead of 16.8MB f32); the residual is
    added host-side from the exact f32 x, so fp8 noise only enters the
    attention branch, which is scaled by gamma=0.1 (int5/e5m2/int4
    input encodings were simulated and fail the 2e-2 gate; e4m3 is the
    smallest safe input encoding)
  - the device returns only the normalized pre-w_o attention tensor
    o2 = (g @ beta^T)/Z, quantized to int4 with a per-(row, 512-col
    block) f32 scale and nibble-packed on-device (1.07MB on the wire
    instead of 2.1MB fp8). End-to-end rel err 1.07e-2 vs the 2e-2
    gate (8.7e-3 with fp8 o2). The w_o matmul, gamma scale and
    residual add run host-side
  - per-core results are AllGathered on-device so the host pulls ONE
    replicated shard instead of eight per-core shards (each extra D2H
    pull costs most of a tunnel round trip: 8 parallel 256KB shard
    pulls measured ~70ms slower than one 2MB pull)
  - the PJRT executable is AOT-compiled ONCE and cached; going through
    run_bass_kernel_spmd would re-trace + re-lower + re-ship the NEFF
    every call (~120ms/call). fast_dispatch_compile removes the
    effects-token sync so dispatch is the C++ fast path (~3ms). The
    donated zero output buffers run_bass_via_pjrt uploads per call are
    dropped: every yout element is written, so uninitialized
    custom-call result buffers are fine
  - the two host hot loops run as AVX2/AVX-512 C via ctypes (compiled
    at first call, numpy fallback): f32->fp8 cast 35ms -> ~5ms, and a
    fused int4-decode + w_og-matmul + residual-add 30ms -> ~12ms.
    This matters doubly because the single vCPU is shared with the
    tunnel client's (de)serialization threads

Device schedule (per core, 2 batch elements; ~90us simulated, ACT
exp-roofline-bound — see _body comments). Sim time is irrelevant to
wall-clock here; it hides entirely under the tunnel round trip.
"""

import os
import sys

import numpy as np

os.environ.setdefault("JAX_PLATFORMS", "axon,cpu")
# smaller NEFF to ship on first compile (debug info is never read here)
os.environ.setdefault("CONCOURSE_SCRUB_NEFF_DEBUG_INFO", "1")
for _p in ("/opt/trn_rl_repo",):
    if _p not in sys.path:
        sys.path.insert(0, _p)

import jax
import concourse.bacc as bacc
import concourse.tile as tile
from concourse import mybir
from concourse import bass2jax

F32 = mybir.dt.float32
BF16 = mybir.dt.bfloat16
F8 = mybir.dt.float8e4
U8 = mybir.dt.uint8
AX = mybir.AluOpType
EXP = mybir.ActivationFunctionType.Exp
BF16_NP = mybir.dt.np(mybir.dt.bfloat16)
F8_NP = mybir.dt.np(mybir.dt.float8e4)

N_CORES = 8
NB = 2          # batch elements per core
C = 64
S = 4096        # H*W
T = 1024        # pooled spatial
SB = 512        # s-block width
NSB = S // SB   # 8
NTC = T // 128  # 8 t-chunks
GROUPS = [(0, 2), (2, 5), (5, 8)]  # t-chunk grouping for big ACT exp ops
PB = SB // 2      # packed bytes per s-block (2 int4 / byte)
Q = 7.49          # int4 quant: u = round(o2*Q/amax + Q) in [0, 15]
ROW = NSB * PB + 4 * NSB  # 2048 packed bytes + 8 f32 scales per row

_cache = {}
last_results = None

# C helpers for the two host-side hot loops (1 vCPU, numpy is ~4x slower):
# f32 -> fp8e4m3 input cast (F16C convert + 64K LUT) and int4 output decode
# (byte -> two scaled f32). Compiled on first use; numpy fallback if cc or
# the compile is unavailable.
_C_SRC = r"""
#include <stdint.h>
#include <immintrin.h>

void cast_f32_to_f8(const float *x, const uint8_t *lut, uint8_t *out,
                    long n) {
    long i = 0;
    for (; i + 8 <= n; i += 8) {
        __m256 v = _mm256_loadu_ps(x + i);
        __m128i h = _mm256_cvtps_ph(v, _MM_FROUND_TO_NEAREST_INT);
        uint16_t tmp[8];
        _mm_storeu_si128((__m128i *)tmp, h);
        out[i + 0] = lut[tmp[0]];
        out[i + 1] = lut[tmp[1]];
        out[i + 2] = lut[tmp[2]];
        out[i + 3] = lut[tmp[3]];
        out[i + 4] = lut[tmp[4]];
        out[i + 5] = lut[tmp[5]];
        out[i + 6] = lut[tmp[6]];
        out[i + 7] = lut[tmp[7]];
    }
    for (; i < n; i++) {
        uint16_t h = _cvtss_sh(x[i], _MM_FROUND_TO_NEAREST_INT);
        out[i] = lut[h];
    }
}

/* raw: rows x rowbytes, each row = nblk*pb packed bytes then nblk f32
   amax scales; lutpair: 256 pairs of (hi - Q, lo - Q); out: rows x
   (nblk*pb*2) floats, scaled by amax/Q per block. */
void decode_int4(const uint8_t *raw, const float *lutpair, float *out,
                 long rows, long nblk, long pb, long rowbytes, float inv_q) {
    for (long r = 0; r < rows; r++) {
        const uint8_t *prow = raw + r * rowbytes;
        const float *amax = (const float *)(prow + nblk * pb);
        float *orow = out + r * nblk * pb * 2;
        for (long j = 0; j < nblk; j++) {
            float s = amax[j] * inv_q;
            const uint8_t *p = prow + j * pb;
            float *o = orow + j * pb * 2;
            for (long i = 0; i < pb; i++) {
                const float *pair = lutpair + 2 * p[i];
                o[2 * i] = pair[0] * s;
                o[2 * i + 1] = pair[1] * s;
            }
        }
    }
}

/* Fused int4 decode -> (w_og @ o2) -> + x residual.
   raw: [B][CH][rowbytes] device output (packed int4 + per-block scales)
   w_og: [OC][CH], x/out: [B][OC][nblk*pb*2] f32. out = w_og@o2 + x. */
void post_all(const uint8_t *raw, const float *lutpair, const float *w_og,
              const float *x, float *out, long B, long CH, long OC,
              long nblk, long pb, long rowbytes, float inv_q) {
    long S = nblk * pb * 2;
    long bw = pb * 2; /* block width in floats (1024 halves? no: pb*2) */
    float vals[32 * 1024] __attribute__((aligned(32)));
    for (long b = 0; b < B; b++) {
        const uint8_t *rb = raw + b * CH * rowbytes;
        for (long j = 0; j < nblk; j++) {
            for (long c = 0; c < CH; c++) {
                const uint8_t *prow = rb + c * rowbytes;
                const float *amax = (const float *)(prow + nblk * pb);
                float s = amax[j] * inv_q;
                const uint8_t *p = prow + j * pb;
                float *v = vals + c * bw;
                for (long i = 0; i < pb; i++) {
                    const float *pair = lutpair + 2 * p[i];
                    v[2 * i] = pair[0] * s;
                    v[2 * i + 1] = pair[1] * s;
                }
            }
            for (long o = 0; o < OC; o += 4) {
                const float *w0 = w_og + o * CH;
                const float *w1 = w_og + (o + 1) * CH;
                const float *w2 = w_og + (o + 2) * CH;
                const float *w3 = w_og + (o + 3) * CH;
                const float *xr = x + (b * OC + o) * S + j * bw;
                float *orow = out + (b * OC + o) * S + j * bw;
#ifdef __AVX512F__
                for (long n = 0; n < bw; n += 16) {
                    __m512 a0 = _mm512_loadu_ps(xr + n);
                    __m512 a1 = _mm512_loadu_ps(xr + S + n);
                    __m512 a2 = _mm512_loadu_ps(xr + 2 * S + n);
                    __m512 a3 = _mm512_loadu_ps(xr + 3 * S + n);
                    for (long c = 0; c < CH; c++) {
                        __m512 v = _mm512_loadu_ps(vals + c * bw + n);
                        a0 = _mm512_fmadd_ps(_mm512_set1_ps(w0[c]), v, a0);
                        a1 = _mm512_fmadd_ps(_mm512_set1_ps(w1[c]), v, a1);
                        a2 = _mm512_fmadd_ps(_mm512_set1_ps(w2[c]), v, a2);
                        a3 = _mm512_fmadd_ps(_mm512_set1_ps(w3[c]), v, a3);
                    }
                    _mm512_storeu_ps(orow + n, a0);
                    _mm512_storeu_ps(orow + S + n, a1);
                    _mm512_storeu_ps(orow + 2 * S + n, a2);
                    _mm512_storeu_ps(orow + 3 * S + n, a3);
                }
#else
                for (long n = 0; n < bw; n += 8) {
                    __m256 a0 = _mm256_loadu_ps(xr + n);
                    __m256 a1 = _mm256_loadu_ps(xr + S + n);
                    __m256 a2 = _mm256_loadu_ps(xr + 2 * S + n);
                    __m256 a3 = _mm256_loadu_ps(xr + 3 * S + n);
                    for (long c = 0; c < CH; c++) {
                        __m256 v = _mm256_loadu_ps(vals + c * bw + n);
                        a0 = _mm256_fmadd_ps(_mm256_set1_ps(w0[c]), v, a0);
                        a1 = _mm256_fmadd_ps(_mm256_set1_ps(w1[c]), v, a1);
                        a2 = _mm256_fmadd_ps(_mm256_set1_ps(w2[c]), v, a2);
                        a3 = _mm256_fmadd_ps(_mm256_set1_ps(w3[c]), v, a3);
                    }
                    _mm256_storeu_ps(orow + n, a0);
                    _mm256_storeu_ps(orow + S + n, a1);
                    _mm256_storeu_ps(orow + 2 * S + n, a2);
                    _mm256_storeu_ps(orow + 3 * S + n, a3);
                }
#endif
            }
        }
    }
}
"""


def _build_chelper():
    import ctypes
    import subprocess
    import tempfile

    try:
        d = tempfile.mkdtemp(prefix="k_chelp_")
        src = os.path.join(d, "helper.c")
        so = os.path.join(d, "helper.so")
        with open(src, "w") as f:
            f.write(_C_SRC)
        subprocess.run(
            ["cc", "-O3", "-march=native", "-shared", "-fPIC", "-o", so, src],
            check=True, capture_output=True, timeout=120,
        )
        lib = ctypes.CDLL(so)
        lib.cast_f32_to_f8.argtypes = [
            ctypes.c_void_p, ctypes.c_void_p, ctypes.c_void_p, ctypes.c_long]
        lib.decode_int4.argtypes = [
            ctypes.c_void_p, ctypes.c_void_p, ctypes.c_void_p,
            ctypes.c_long, ctypes.c_long, ctypes.c_long, ctypes.c_long,
            ctypes.c_float]
        lib.post_all.argtypes = [
            ctypes.c_void_p, ctypes.c_void_p, ctypes.c_void_p, ctypes.c_void_p,
            ctypes.c_void_p, ctypes.c_long, ctypes.c_long, ctypes.c_long,
            ctypes.c_long, ctypes.c_long, ctypes.c_long, ctypes.c_float]
        return lib
    except Exception:
        return None


def _build_program():
    nc = bacc.Bacc(None, target_bir_lowering=False, debug=False, num_devices=N_CORES)
    xin = nc.dram_tensor("xin", [NB, C, S], F8, kind="ExternalInput")
    # cols 0:96 = fused conv weights; rows 0:32 of cols 96:128 = identity
    wcat = nc.dram_tensor("wcat", [C, 128], BF16, kind="ExternalInput")
    # per row: 2048 bytes of nibble-packed int4 o2 + 8 f32 block scales
    yout = nc.dram_tensor("yout", [N_CORES, NB, 32, ROW], U8, kind="ExternalOutput")

    with tile.TileContext(nc) as tc:
        with nc.allow_low_precision(reason="bf16 attention; residual is f32 host-side"):
            _body(tc, xin, wcat, yout)
    nc.compile()
    return nc


def _body(tc, xin, wcat, yout):
    nc = tc.nc
    with (
        tc.tile_pool(name="const", bufs=1) as cpool,
        tc.tile_pool(name="big", bufs=2) as bpool,
        tc.tile_pool(name="work", bufs=2) as wpool,
        tc.tile_pool(name="stexp", bufs=4) as epool,
        tc.tile_pool(name="dram", bufs=1, space="DRAM") as dpool,
        tc.psum_pool(name="ps_sc", bufs=2) as ps_sc,
        tc.psum_pool(name="ps_o", bufs=2) as ps_o,
    ):
        # per-core result staged in internal DRAM, AllGathered to every
        # core's ExternalOutput so the host fetches ONE shard instead of
        # eight per-core shards (each extra D2H pull costs ~a tunnel
        # roundtrip)
        ylocal = dpool.tile([NB, 32, ROW], U8)
        ybounce = dpool.tile([N_CORES, NB, 32, ROW], U8)
        wcat_sb = cpool.tile([C, 128], BF16)
        nc.sync.dma_start(wcat_sb[:], wcat[:])
        ident_sb = wcat_sb[0:32, 96:128]
        ones_f = cpool.tile([128, 1], F32)
        nc.vector.memset(ones_f[:], 1.0)
        # warm-up exp on a scalar so the framework emits LoadActFuncSet at
        # the head of the ACT queue (during the input DMA) instead of lazily
        # right before the first real exp ~8us in
        act_warm = cpool.tile([1, 1], F32)
        nc.scalar.activation(act_warm[:], ones_f[0:1, 0:1], EXP)

        # dummy custom-DVE op (output unused): routes DVE table generation
        # through the process-cached dve_table_for_ops path (~0.3s/compile
        # saved). Emitted via a closure after batch 0's conv so it does not
        # sit at the head of the DVE queue.
        def dve_dummy_op():
            dve_dummy = cpool.tile([1, 1], F32)
            nc.vector.reciprocal_approx_fast(dve_dummy[:], ones_f[0:1, 0:1])

        state = {}

        def p1_start(b):
            """input DMA (group-aligned slices) + fp8->bf16 casts + tile
            allocation for batch b. Cast g follows its own DMA slice; casts
            alternate DVE/GpSimd so no cast is queue-blocked by copies."""
            x8_sb = bpool.tile([C, S], F8, tag="x8")
            x_sb = bpool.tile([C, S], BF16, tag="x")
            pre_sb = bpool.tile([96, S], BF16, tag="pre")
            phm = wpool.tile([8, 2048], BF16, tag="phm")
            phi_sb = wpool.tile([8, T], BF16, tag="phi")
            for gi, (g0, g1) in enumerate(GROUPS):
                nc.sync.dma_start(
                    x8_sb[:, g0 * SB:g1 * SB], xin[b][:, g0 * SB:g1 * SB])
                # batch 1's casts all ride GpSimd: its phi chain competes
                # with batch 0's steady-state DVE work (recip/mult)
                eng = nc.gpsimd if (b == 1 or gi == 1) else nc.vector
                eng.tensor_copy(
                    x_sb[:, g0 * SB:g1 * SB], x8_sb[:, g0 * SB:g1 * SB])
            scales_sb = bpool.tile([32, NSB], F32, tag="scales")
            state[b] = {"x8": x8_sb, "x": x_sb, "pre": pre_sb, "phm": phm,
                        "phi": phi_sb, "scales": scales_sb}

        def p1_conv(b, gi):
            """conv group gi for batch b: 3 matmuls -> copies.
            theta+phi rows copy on DVE (feeds pools/scores); g rows on GpSimd.
            The h-direction phi maxpool runs per group right after its copy."""
            st = state[b]
            x_sb, pre_sb, phm = st["x"], st["pre"], st["phm"]
            g0, g1 = GROUPS[gi]
            cps = ps_sc.tile([96, (g1 - g0) * SB], F32, tag="sc")
            for j in range(g0, g1):
                nc.tensor.matmul(
                    cps[:, (j - g0) * SB:(j - g0 + 1) * SB],
                    wcat_sb[:, 0:96], x_sb[:, j * SB:(j + 1) * SB],
                    start=True, stop=True,
                )
            if b == 0:
                # b0: phi-critical rows drain on DVE, g rows on ACT (idle
                # during startup; GpSimd can't read PSUM) so g2t can start
                # early
                nc.vector.tensor_copy(
                    pre_sb[0:40, g0 * SB:g1 * SB], cps[0:40, :])
                nc.scalar.activation(
                    pre_sb[64:96, g0 * SB:g1 * SB], cps[64:96, :],
                    mybir.ActivationFunctionType.Copy)
            else:
                # b1: one full-width DVE drain (same free-dim cost as the
                # 40-row copy) keeps its Copies out of the mid-stream ACT
                # queue; b1's g-path has slack so nothing needs them early
                nc.vector.tensor_copy(
                    pre_sb[:, g0 * SB:g1 * SB], cps[:])
            # phi h-max for this group's columns (cols are (h w) pairs in w)
            nrow = (g1 - g0) * SB // 64  # 64-wide w rows in this slice
            pv = pre_sb[32:40, g0 * SB:g1 * SB].rearrange(
                "p (h w) -> p h w", h=nrow)
            nc.vector.tensor_tensor(
                phm[:, g0 * SB // 2:g1 * SB // 2].rearrange(
                    "p (h w) -> p h w", h=nrow),
                pv[:, :, 0:64:2], pv[:, :, 1:64:2], AX.max)
            # phi w-max for the same slice: yields phi t-chunks [g0*128,
            # g1*128), exactly the score chunks this group's exps will read,
            # so the first scores can start after conv group 0 alone
            phi_sb = st["phi"]
            ph2 = phm[:, g0 * SB // 2:g1 * SB // 2].rearrange(
                "p (h w) -> p h w", h=nrow)
            nc.vector.tensor_tensor(
                phi_sb[:, g0 * 128:g1 * 128].rearrange(
                    "p (h w) -> p h w", h=nrow // 2),
                ph2[:, 0:nrow:2, :], ph2[:, 1:nrow:2, :], AX.max)

        def p1_gpools(b):
            """g maxpool (DVE) — only gates g2t, emitted off the scores path."""
            st = state[b]
            pre_sb = st["pre"]
            g_sb = wpool.tile([32, T], BF16, tag="g")
            ghm = wpool.tile([32, 2048], BF16, tag="ghm")
            gv = pre_sb[64:96].rearrange("p (h w) -> p h w", h=64)
            nc.vector.tensor_tensor(
                ghm[:].rearrange("p (h w) -> p h w", h=64),
                gv[:, :, 0:64:2], gv[:, :, 1:64:2], AX.max)
            gh2 = ghm[:].rearrange("p (h w) -> p h w", h=64)
            nc.vector.tensor_tensor(
                g_sb[:].rearrange("p (h w) -> p h w", h=32),
                gh2[:, 0:64:2, :], gh2[:, 1:64:2, :], AX.max)
            st["g"] = g_sb

        def phase1_g2t(b):
            """g2T chunks: [128 t, 33] = g[:, chunk].T via identity; col 32 =
            ones. Emitted after the first scores block of batch b so the PE
            queue starts scores as soon as phi is pooled."""
            g_sb = state[b]["g"]
            g2t_sb = bpool.tile([128, NTC * 33], BF16, tag="g2t")
            nc.gpsimd.tensor_copy(
                g2t_sb[:].rearrange("p (k c) -> p k c", c=33)[:, :, 32],
                ones_f[:].to_broadcast([128, NTC]))
            for k in range(NTC):
                g2ps = ps_o.tile([128, 32], F32, tag="o")
                nc.tensor.matmul(
                    g2ps[:], g_sb[:, k * 128:(k + 1) * 128], ident_sb[:],
                    start=True, stop=True,
                )
                nc.vector.tensor_copy(g2t_sb[:, k * 33:k * 33 + 32], g2ps[:])
            state[b]["g2t"] = g2t_sb

        def p2_scores(j, b):
            """scores -> exp for (j, b). One st_exp tile per exp group so
            the o-matmul's per-chunk reads depend only on their own group's
            exp, not all three."""
            pre_sb, phi_sb = state[b]["pre"], state[b]["phi"]
            theta = pre_sb[0:8]
            st_exp = []
            for gi, (k0, k1) in enumerate(GROUPS):
                scps = ps_sc.tile([128, 3 * SB], F32, tag="sc")
                for k in range(k0, k1):
                    nc.tensor.matmul(
                        scps[:, (k - k0) * SB:(k - k0 + 1) * SB],
                        phi_sb[:, k * 128:(k + 1) * 128],
                        theta[:, j * SB:(j + 1) * SB],
                        start=True, stop=True,
                    )
                se = epool.tile([128, (k1 - k0) * SB], BF16, tag=f"se{gi}")
                nc.scalar.activation(se[:], scps[:, 0:(k1 - k0) * SB], EXP)
                st_exp.append(se)
            return st_exp

        def p2_rest(j, b, st_exp):
            """o-matmul -> normalize -> int4 quantize+pack -> DMA of (j, b)."""
            g2t_sb = state[b]["g2t"]
            o_ps = ps_o.tile([33, SB], F32, tag="o")
            for k in range(NTC):
                gi = 0 if k < 2 else (1 if k < 5 else 2)
                kk = k - GROUPS[gi][0]
                nc.tensor.matmul(
                    o_ps[:],
                    g2t_sb[:, k * 33:(k + 1) * 33],
                    st_exp[gi][:, kk * SB:(kk + 1) * SB],
                    start=(k == 0), stop=(k == NTC - 1),
                )

            # normalize straight out of PSUM (no staging copy): the "o" ring
            # slot stays held until the mult reads it, which is still well
            # before the next-but-one o-matmul needs the bank. 1/Z fans out
            # across the 32 channel partitions on the GpSimd engine so the
            # mult has a single PSUM operand.
            zr = wpool.tile([1, SB], BF16, tag="zr")
            nc.vector.reciprocal(zr[:], o_ps[32:33, :])
            zb_sb = wpool.tile([32, SB], BF16, tag="zb")
            nc.gpsimd.partition_broadcast(zb_sb[:], zr[:])
            o_f = wpool.tile([32, SB], F32, tag="of")
            nc.vector.tensor_tensor(o_f[:], o_ps[0:32, :], zb_sb[:], AX.mult)
            # int4 quantize with per-(row, block) scale: u = o*Q/amax + Q
            # rounds into [0, 15]; amax=0 rows decode to 0 via the host-side
            # amax multiply, so no special-casing beyond the 1e-6 clamp
            amax = wpool.tile([32, 1], F32, tag="amax")
            nc.vector.tensor_reduce(
                amax[:], o_f[:], mybir.AxisListType.X, AX.max,
                apply_absolute_value=True)
            nc.vector.tensor_scalar_max(amax[:], amax[:], 1e-6)
            rcp = wpool.tile([32, 1], F32, tag="rcp")
            nc.vector.reciprocal(rcp[:], amax[:])
            rsc = wpool.tile([32, 1], F32, tag="rsc")
            nc.vector.tensor_scalar_mul(rsc[:], rcp[:], Q)
            u8 = wpool.tile([32, SB], U8, tag="u8")
            nc.scalar.activation(
                u8[:], o_f[:], mybir.ActivationFunctionType.Copy,
                bias=Q, scale=rsc[:])
            # nibble-pack adjacent columns: byte i = u[2i]*16 + u[2i+1]
            hi = wpool.tile([32, PB], U8, tag="hi")
            nc.vector.tensor_scalar_mul(hi[:], u8[:, 0:SB:2], 16)
            pk = wpool.tile([32, PB], U8, tag="pk")
            nc.vector.tensor_tensor(pk[:], hi[:], u8[:, 1:SB:2], AX.add)
            nc.sync.dma_start(ylocal[b][:, j * PB:(j + 1) * PB], pk[:])
            nc.vector.tensor_copy(state[b]["scales"][:, j:j + 1], amax[:])

        # staggered schedule: batch 0's first scores start as early as
        # possible; g2t and batch 1's conv groups ride in the exp shadow of
        # batch 0's early j-blocks; then (j, b) pairs alternate so
        # PE/ACT/DVE/GpSimd stay fed
        p1_start(0)
        for gi in range(3):
            p1_conv(0, gi)
        se00 = p2_scores(0, 0)
        dve_dummy_op()
        p1_start(1)
        p1_conv(1, 0)
        se10 = p2_scores(1, 0)
        p1_conv(1, 1)
        p1_conv(1, 2)
        p1_gpools(0)
        phase1_g2t(0)
        p2_rest(0, 0, se00)
        se20 = p2_scores(2, 0)
        p1_gpools(1)
        p2_rest(1, 0, se10)
        phase1_g2t(1)

        order = [(0, 1)]
        for j in range(3, NSB):
            order.append((j, 0))
            order.append((j - 2, 1))
        order.append((NSB - 2, 1))
        order.append((NSB - 1, 1))
        # two-deep software pipeline: scores run ahead of the o-matmuls so
        # the PE queue always has the next blocks' scores ready, keeping
        # ACT's exp stream gapless (st_exp rings hold the blocks in flight)
        from collections import deque
        pend = deque([(2, 0, se20)])
        for (j, b) in order[:-1]:
            se = p2_scores(j, b)
            pend.append((j, b, se))
            if len(pend) > 2:
                p2_rest(*pend.popleft())
        jl, bl = order[-1]
        sel = p2_scores(jl, bl)
        while pend:
            p2_rest(*pend.popleft())
        p2_rest(jl, bl, sel)

        # per-batch block scales ride in-band after the packed bytes
        for b in range(NB):
            nc.sync.dma_start(
                ylocal[b][:, NSB * PB:ROW], state[b]["scales"][:].bitcast(U8))

        # gather every core's result so core 0 holds the full batch
        nc.gpsimd.collective_compute(
            "AllGather",
            mybir.AluOpType.bypass,
            replica_groups=[list(range(N_CORES))],
            ins=[ylocal.opt()],
            outs=[ybounce.opt()],
        )
        nc.sync.dma_start(yout[:], ybounce[:])


def _build_executable():
    """AOT-compile the sharded PJRT executable once.

    Bypasses run_bass_kernel_spmd, which re-traces, re-lowers and re-ships
    the NEFF on every call (~120ms/call through the axon tunnel). The
    donated zero output buffers it uploads each call are also dropped: the
    kernel writes every element of yout, so uninitialized custom-call
    result buffers are fine.
    """
    from jax.sharding import Mesh, PartitionSpec
    from jax.experimental.shard_map import shard_map

    nc = _build_program()
    bass2jax.install_neuronx_cc_hook()
    partition_name = nc.partition_id_tensor.name if nc.partition_id_tensor else None
    out_aval = jax.core.ShapedArray((N_CORES, NB, 32, ROW), np.uint8)
    in_names = ["xin", "wcat"] + ([partition_name] if partition_name else [])

    def _exec_body(xin, wcat):
        operands = [xin, wcat]
        if partition_name is not None:
            operands.append(bass2jax.partition_id_tensor())
        outs = bass2jax._bass_exec_p.bind(
            *operands,
            out_avals=(out_aval,),
            in_names=tuple(in_names),
            out_names=("yout",),
            lowering_input_output_aliases=(),
            sim_require_finite=True,
            sim_require_nnan=True,
            nc=nc,
        )
        return outs[0]

    devices = jax.devices()[:N_CORES]
    mesh = Mesh(np.asarray(devices), ("core",))
    sharded = shard_map(
        _exec_body,
        mesh=mesh,
        in_specs=(PartitionSpec("core"), PartitionSpec("core")),
        # the on-device AllGather makes yout identical on every core; P()
        # marks it replicated so np.asarray pulls from a single shard
        out_specs=PartitionSpec(),
        check_rep=False,
    )
    xin_tmpl = jax.ShapeDtypeStruct((N_CORES * NB, C, S), F8_NP)
    wcat_tmpl = jax.ShapeDtypeStruct((N_CORES * C, 128), BF16_NP)
    return bass2jax.fast_dispatch_compile(
        lambda: jax.jit(sharded).lower(xin_tmpl, wcat_tmpl).compile()
    )


def _get_cached():
    if "exe" not in _cache:
        _cache["exe"] = _build_executable()
        # packed byte -> (hi, lo) int4 value pairs, bias pre-subtracted; the
        # numpy gather is the fastest decode on this 1-vCPU host
        b = np.arange(256, dtype=np.uint8)
        _cache["lut4"] = np.stack(
            [(b >> 4).astype(np.float32) - Q, (b & 15).astype(np.float32) - Q],
            axis=1,
        )
        # f16 -> fp8e4m3 cast table: f32->f16 (SIMD) + byte gather is much
        # faster than ml_dtypes' direct f32->fp8 on this host; the rare
        # double-rounding ties (0.4% of values, 1 ulp) are noise here
        with np.errstate(invalid="ignore"):
            _cache["lut_f8"] = (
                np.arange(65536, dtype=np.uint16).view(np.float16)
                .astype(np.float32).astype(F8_NP)
            )
        _cache["clib"] = _build_chelper()
    return _cache["exe"], _cache["lut4"], _cache["lut_f8"], _cache["clib"]


def kernel(x, w_theta, w_phi, w_g, w_o, gamma):
    global last_results
    last_results = None
    exe, lut4, lut_f8, clib = _get_cached()

    x = np.ascontiguousarray(np.asarray(x, dtype=np.float32)).reshape(16, C, S)
    if clib is not None:
        x_f8 = np.empty((16, C, S), F8_NP)
        clib.cast_f32_to_f8(
            x.ctypes.data, lut_f8.ctypes.data, x_f8.ctypes.data, x.size)
    else:
        x_f8 = lut_f8[x.astype(np.float16).view(np.uint16)]

    wcat_full = np.zeros((128, C), dtype=np.float32)
    wcat_full[0:8] = np.asarray(w_theta)
    wcat_full[32:40] = np.asarray(w_phi)
    wcat_full[64:96] = np.asarray(w_g)
    wcat_full[96:128, 0:32] = np.eye(32, dtype=np.float32)
    wcat_1 = np.ascontiguousarray(wcat_full.T).astype(BF16_NP)
    wcat_np = np.ascontiguousarray(
        np.broadcast_to(wcat_1, (N_CORES, C, 128))
    ).reshape(N_CORES * C, 128)
    w_og = np.ascontiguousarray(
        (float(np.asarray(gamma)) * np.asarray(w_o)).astype(np.float32))

    out = exe(x_f8, wcat_np)
    # pull the single replicated shard (one D2H round trip)
    raw = np.asarray(out.addressable_shards[0].data).reshape(16, 32, ROW)

    # decode int4 o2 (byte i of block j -> cols (2i, 2i+1); scale per
    # block), then out = gamma*(w_o @ o2) + x
    if clib is not None:
        res = np.empty((16, C, S), np.float32)
        clib.post_all(
            raw.ctypes.data, lut4.ctypes.data, w_og.ctypes.data,
            x.ctypes.data, res.ctypes.data, 16, 32, C, NSB, PB, ROW, 1.0 / Q)
    else:
        amax = np.ascontiguousarray(raw[:, :, NSB * PB:]).view(np.float32)
        o2f = lut4[raw[:, :, :NSB * PB]].reshape(16, 32, NSB, SB)
        o2f *= (amax * (1.0 / Q))[..., None]
        res = np.matmul(w_og, o2f.reshape(16, 32, S))
        res += x
    return res.reshape(16, C, 64, 64)
